# revision 15
# baseline (speedup 1.0000x reference)
"""Trainium2 Bass kernel for nn_CrossAttentionTemporal3D (v2).

Sharding: batch x head-pair across 8 cores (core c -> batch c//4, heads
{2*(c%4), 2*(c%4)+1}).  Each core computes q/k/v projections for its two
heads, per-frame spatial attention (frames 1..15) and frame-0 temporal
attention, then the out-projection partial product for its 128 hc
columns.  Host sums the 4 partial outputs per batch and adds bout.

v3 structural changes vs the 317us baseline:
  * QK emitted chunk-inner (per key tile: c0h0, c0h1, c1h0, c1h1), and a
    post-schedule pass NoOps redundant InstLdweights: a load whose
    (weights AP, tile_position) is already resident in its row strips
    with no intervening conflicting load.  Halves QK weight-load
    traffic (LDW ~96ns was pacing the PE against 120ns streams).
  * Norm: the two Ln reads stay per-av-tile, but the two Exp(-x) merge
    into ONE 288-col pass over the shared lnT tile (the Lns write
    disjoint partition halves).  ACT per chunk 1764ns -> ~1190ns.
  * Temporal attention split into 3 sub-units of 3 key frames each,
    placed between spatial units so its exp-heavy (ACT-bound) work is
    spread across the kernel instead of bursting.  Sub-unit partial
    sums are staged to SBUF and merged with PSUM RMW adds before the
    final norm.
  * Projections flow 1-per-unit through position 14 (instead of 1/unit
    then a 5-frame burst at the temporal unit), keeping the PE busy in
    the tail so the HAM clock gate stays at 2.4 GHz (baseline lost
    ~39us to K=4/8 oscillation after t=210us).
  * Frame 1 is projected first so attention/ACT work starts ~2us in.
  * PSUM layout as baseline: lg 2x[128,2,512] (4 banks) + ao rotation
    4x[128,512] (avA/avB/pp/op share 4 slots).

Token layout on device: frame-major (token = f*576 + s) with frames
permuted so the temporal key frames [0, 1, kept...] come first.  Host
pre-transposes x to xT [512, 9216] bf16 in that order.
"""

import sys
import types

for _p in (
    "/root/.axon_site",
    "/root/.axon_site/_ro/trn_rl_repo",
    "/root/.axon_site/_ro/pypackages",
    "/opt/trn_rl_repo",
    "/opt/pypackages",
):
    if _p not in sys.path:
        sys.path.append(_p)

import numpy as np

import concourse.bass as bass
import concourse.tile as tile
from concourse import mybir

F32 = mybir.dt.float32
BF16 = mybir.dt.bfloat16

B, S, F, D = 2, 576, 16, 512
H, C = 8, 64
NT = S * F          # 9216 tokens per batch (frame-major)
NKT = 5             # key tiles per frame: 4 full + one 64-tail
KW = [128, 128, 128, 128, 64]
KOFF = [0, 128, 256, 384, 512]
QCH = [(0, 288), (288, 288)]  # query chunks (offset, len)
VTW = 192           # V cols per key tile: [v_h0(64) | 1*64 | v_h1(64)]
LN288 = 288


def _ap_with_free(ap, free_dims):
    """Clone an AP keeping its partition dim, replacing the free dims."""
    return bass.AP(tensor=ap.tensor, offset=ap.offset, ap=[ap.ap[0]] + free_dims)


_WAIT_LIMITS = {k: 1 for k in ("Drain", "Matmult", "Ldweights", "NoOp", "DMACopy", "Activation", "TensorCopy", "TensorTensor", "TensorScalar", "Memset", "ISA", "TensorReduce", "Reciprocal", "DMATransposeAnt", "DmaTransposeAnt", "InstISA")}


def _split_drain_waits(nc):
    """Walrus allows a single sync wait on several opcodes (TPB_CTRL /
    fused S3_LW weight-load).  Hoist extra waits onto one-wait NoOps
    emitted just before the instruction on the same engine."""
    for bb in nc.main_func.blocks:
        new_list, changed = [], False
        for ins in list(bb.instructions):
            si = getattr(ins, "sync_info", None)
            limit = _WAIT_LIMITS.get(ins.opcode)
            if limit is not None and si is not None and len(si.on_wait) > limit:
                waits = list(si.on_wait)
                for i, w in enumerate(waits[limit:]):
                    nop = mybir.InstNoOp(
                        name=f"{ins.name}-wsplit{i}",
                        engine=ins.engine,
                        sync_info=mybir.SyncInfo(on_wait=[w], on_update=[]),
                        bass_nofuse=True,
                    )
                    nc.register_instruction(nop, overwrite=True)
                    new_list.append(nop)
                si.on_wait = waits[:limit]
                changed = True
            new_list.append(ins)
        if changed:
            bb.instructions[:] = new_list


def _dedup_ldweights(nc):
    """NoOp redundant InstLdweights in the scheduled stream: a load whose
    (weights AP, tile_position, tile_size) already sits in its row strips
    with no intervening overlapping load.  The PE array keeps disjoint
    row-group stationaries resident simultaneously (tile_position), so
    the chunk-inner QK pattern [ldwA mm ldwB mm ldwA' mm ldwB' mm]
    re-loads identical weights; ldwA'/ldwB' become NoOps (later bacc
    passes fuse/remove waitless NoOps entirely)."""
    n = 0
    for bb in nc.main_func.blocks:
        resident = []  # list of (row_lo, row_hi, sig)
        new_list = []
        for ins in list(bb.instructions):
            if ins.opcode == "Ldweights":
                tp = ins.tile_position or (0, 0)
                ts = ins.tile_size
                rows = ts[0] if ts else 128
                lo, hi = tp[0], tp[0] + rows
                sig = (str(ins.ins[0]), tuple(tp), tuple(ts) if ts else None,
                       ins.perf_mode, ins.is_transpose)
                hit = any(lo == rl and hi == rh and sig == rs
                          for rl, rh, rs in resident)
                if hit:
                    nop = mybir.InstNoOp(
                        name=ins.name,
                        engine=ins.engine,
                        sync_info=ins.sync_info,
                        bass_nofuse=True,
                    )
                    nc.register_instruction(nop, overwrite=True)
                    new_list.append(nop)
                    n += 1
                    continue
                resident = [(rl, rh, rs) for rl, rh, rs in resident
                            if rh <= lo or rl >= hi]
                resident.append((lo, hi, sig))
            new_list.append(ins)
        bb.instructions[:] = new_list
    return n


def _weave(streams):
    """Interleave quanta lists proportionally (lowest emitted-fraction
    first), preserving each stream's internal order."""
    streams = [s for s in streams if s]
    idx = [0] * len(streams)
    total = sum(len(s) for s in streams)
    for _ in range(total):
        best, bf = -1, 10.0
        for i, s in enumerate(streams):
            if idx[i] < len(s):
                f = (idx[i] + 1.0) / len(s)
                if f < bf:
                    bf, best = f, i
        streams[best][idx[best]]()
        idx[best] += 1


def build_program(G):
    """Build the per-core Bass program. G = number of temporal key frames."""
    nc = bass.Bass()
    xT = nc.dram_tensor("xT", [D, NT], BF16, kind="ExternalInput")
    wq = nc.dram_tensor("wq", [D, 128], BF16, kind="ExternalInput")
    wk = nc.dram_tensor("wk", [D, 128], BF16, kind="ExternalInput")
    wv = nc.dram_tensor("wv", [D, 128], BF16, kind="ExternalInput")
    wout = nc.dram_tensor("wout", [128, D], BF16, kind="ExternalInput")
    out = nc.dram_tensor("out", [NT, D], F32, kind="ExternalOutput")

    NOSYNC = mybir.DependencyInfo.NO_SYNC_ONLY
    EXP = mybir.ActivationFunctionType.Exp
    LN = mybir.ActivationFunctionType.Ln
    MULT = mybir.AluOpType.mult
    ADD = mybir.AluOpType.add

    from contextlib import ExitStack

    with tile.TileContext(nc) as tc, ExitStack() as ctx:
        consts = ctx.enter_context(tc.tile_pool(name="consts", bufs=1))
        big = ctx.enter_context(tc.tile_pool(name="big", bufs=1))
        xt_pool = ctx.enter_context(tc.tile_pool(name="xt", bufs=3))
        pt_pool = ctx.enter_context(tc.tile_pool(name="pt", bufs=34))
        resT_pool = ctx.enter_context(tc.tile_pool(name="resT", bufs=3))
        nrm_pool = ctx.enter_context(tc.tile_pool(name="nrm", bufs=2))
        stage_pool = ctx.enter_context(tc.tile_pool(name="stg", bufs=3))
        ptl_pool = ctx.enter_context(tc.tile_pool(name="ptl", bufs=4))
        lg_psum = ctx.enter_context(tc.tile_pool(name="lg", bufs=2, space="PSUM"))
        ao_psum = ctx.enter_context(tc.tile_pool(name="ao", bufs=4, space="PSUM"))

        # ---- frame-1 tokens first: the whole pipeline's critical path
        # starts at proj(1), so its DMA goes ahead of the constants.
        def xt_dma(f):
            # split per dt4 block so the first projection matmul can start
            # as soon as the first quarter lands
            xt = xt_pool.tile([128, 4, S], BF16, tag="xt", name="xt")
            src = xT.rearrange("(a p) n -> p a n", p=128)[:, :, S * f : S * (f + 1)]
            for j in range(4):
                nc.sync.dma_start(out=xt[:, j, :], in_=src[:, j, :])
            return xt

        xt1 = xt_dma(1)

        # ---- constants
        wq_sb = consts.tile([128, 4, 128], BF16)
        wk_sb = consts.tile([128, 4, 128], BF16)
        wv_sb = consts.tile([128, 4, 128], BF16)
        wout_sb = consts.tile([128, 512], BF16)
        nc.sync.dma_start(out=wq_sb, in_=wq.rearrange("(a p) c -> p a c", p=128))
        nc.sync.dma_start(out=wk_sb, in_=wk.rearrange("(a p) c -> p a c", p=128))
        nc.sync.dma_start(out=wv_sb, in_=wv.rearrange("(a p) c -> p a c", p=128))
        nc.sync.dma_start(out=wout_sb, in_=wout[:, :])

        # ---- HAM warm-up: ~4us of dependency-free junk matmuls that run
        # during the initial DMA-wait window (PE idle 6.5-13us otherwise),
        # flipping the clock gate to 2.4GHz before the first projection.
        # Values are garbage and never read; the lg slot is recycled by the
        # pool semaphores before its first real use.
        junk = consts.tile([128, 512], BF16, name="junk")
        nc.gpsimd.memset(junk[:, :], 1.0)
        jp = lg_psum.tile([128, 2, 512], F32, tag="lg", name="jp")
        for _ in range(12):
            nc.tensor.matmul(
                jp[:, 0, :],
                lhsT=junk[:, 0:128],
                rhs=junk[:, :],
                start=True,
                stop=True,
            )

        # ---- persistent activations
        qT = big.tile([128, NT], BF16)   # [2-head c, token]
        kT = big.tile([128, NT], BF16)
        V = big.tile([128, F * NKT * VTW], BF16)
        # ones columns 64:128 of every key tile
        nc.vector.memset(
            _ap_with_free(V[:, 64:65], [[VTW, F * NKT], [1, 64]]), 1.0
        )

        # ------------------------------------------------------------------
        # projection stream for frame f: dma, q/k chunks, v token-tiles
        def proj_quanta(f, pre_xt=None):
            st = {}
            quanta = []

            def dma_q():
                st["xt"] = pre_xt if pre_xt is not None else xt_dma(f)

            quanta.append(dma_q)

            for w_sb, dest in ((wq_sb, qT), (wk_sb, kT)):
                for off, ln in QCH:
                    def qk_q(w_sb=w_sb, dest=dest, off=off, ln=ln):
                        pp = ao_psum.tile([128, 512], F32, tag="ao", name="pp")
                        for dt4 in range(4):
                            nc.tensor.matmul(
                                pp[:, 0:ln],
                                lhsT=w_sb[:, dt4, :],
                                rhs=st["xt"][:, dt4, off : off + ln],
                                start=(dt4 == 0),
                                stop=(dt4 == 3),
                            )
                        nc.vector.tensor_copy(
                            dest[:, S * f + off : S * f + off + ln],
                            pp[:, 0:ln],
                        )

                    quanta.append(qk_q)

            for t in range(NKT):
                def v_q(t=t):
                    w = KW[t]
                    pp = ao_psum.tile([128, 512], F32, tag="ao", name="ppv")
                    for dt4 in range(4):
                        nc.tensor.matmul(
                            pp[0:w, 0:128],
                            lhsT=st["xt"][:, dt4, KOFF[t] : KOFF[t] + w],
                            rhs=wv_sb[:, dt4, :],
                            start=(dt4 == 0),
                            stop=(dt4 == 3),
                        )
                    base = VTW * (NKT * f + t)
                    dst = _ap_with_free(V[0:w, base : base + 1], [[128, 2], [1, 64]])
                    src = _ap_with_free(pp[0:w, 0:1], [[64, 2], [1, 64]])
                    nc.vector.tensor_copy(out=dst, in_=src)

                quanta.append(v_q)
            return quanta

        # ------------------------------------------------------------------
        # attention stream: chunk-inner QK with stationary reuse, AV chunk
        # c0 inline, AV chunk c1 deferred to the unit tail.
        def attn_quanta(q0, key_tiles, finish0, finish1):
            nk = len(key_tiles)
            st = {"pts": {}, "lgs": {}, "av": {}}
            quanta = []

            def mk_qk(t):
                def qk(t=t):
                    vt, koff, w = key_tiles[t]
                    mm_w = 128 if koff + 128 <= NT else w
                    lgA = lg_psum.tile([128, 2, 512], F32, tag="lg", name="lgA")
                    lgB = lg_psum.tile([128, 2, 512], F32, tag="lg", name="lgB")
                    for ci, (off, ln) in enumerate(QCH):
                        lg = (lgA, lgB)[ci]
                        for h in (0, 1):
                            hb = 64 * h
                            nc.tensor.matmul(
                                lg[0:mm_w, h, 0:ln],
                                lhsT=kT[hb : hb + 64, koff : koff + mm_w],
                                rhs=qT[hb : hb + 64, q0 + off : q0 + off + ln],
                                start=True,
                                stop=True,
                                tile_position=(hb, 0),
                            )
                    st["lgs"][t] = (lgA, lgB, mm_w)

                return qk

            def mk_exp(t, ci):
                def ex(t=t, ci=ci):
                    lgA, lgB, mm_w = st["lgs"][t]
                    lg = (lgA, lgB)[ci]
                    pt = pt_pool.tile([128, 2, LN288], BF16, tag="pt", name="pt")
                    nc.scalar.activation(
                        out=pt[0:mm_w, :, :],
                        in_=lg[0:mm_w, :, 0:LN288],
                        func=EXP,
                    )
                    st["pts"][(t, ci)] = pt

                return ex

            def mk_av(ci, t):
                def av_q(ci=ci, t=t):
                    vt, koff, w = key_tiles[t]
                    if t == 0:
                        avA = ao_psum.tile([128, 512], F32, tag="ao", name="avA")
                        avB = ao_psum.tile([128, 512], F32, tag="ao", name="avB")
                        st["av"][ci] = (avA, avB)
                    avA, avB = st["av"][ci]
                    pt = st["pts"].pop((t, ci))
                    base = VTW * vt
                    nc.tensor.matmul(
                        avA[0:128, 0:LN288],
                        lhsT=V[0:w, base : base + 128],
                        rhs=pt[0:w, 0, :],
                        start=(t == 0),
                        stop=(t == nk - 1),
                    )
                    nc.tensor.matmul(
                        avB[0:128, 0:LN288],
                        lhsT=V[0:w, base + 64 : base + VTW],
                        rhs=pt[0:w, 1, :],
                        start=(t == 0),
                        stop=(t == nk - 1),
                    )

                return av_q

            quanta.append(mk_qk(0))
            if nk > 1:
                quanta.append(mk_qk(1))
            for t in range(nk):
                quanta.append(mk_exp(t, 0))
                quanta.append(mk_av(0, t))
                quanta.append(mk_exp(t, 1))
                if t + 2 < nk:
                    quanta.append(mk_qk(t + 2))
            quanta.append(lambda: finish0(st["av"][0]))
            for t in range(nk):
                quanta.append(mk_av(1, t))
            quanta.append(lambda: finish1(st["av"][1]))
            return quanta

        # ---- chunk finishers
        def make_norm_fin(hold, off, parts=None):
            def fin(av):
                avA, avB = av
                if "r" not in hold:
                    hold["r"] = resT_pool.tile(
                        [128, S], BF16, tag="resT", name="resT"
                    )
                resT = hold["r"]
                if parts is not None:
                    for p in parts():
                        nc.vector.tensor_tensor(
                            avA[:, 0:LN288], avA[:, 0:LN288], p[:, 0, :], ADD
                        )
                        nc.vector.tensor_tensor(
                            avB[:, 0:LN288], avB[:, 0:LN288], p[:, 1, :], ADD
                        )
                lnT = nrm_pool.tile([128, LN288], F32, tag="lnT", name="lnT")
                rb = nrm_pool.tile([128, LN288], F32, tag="rb", name="rb")
                nc.scalar.activation(out=lnT[64:128, :], in_=avA[64:128, 0:LN288],
                                     func=LN)
                nc.scalar.activation(out=lnT[0:64, :], in_=avB[0:64, 0:LN288],
                                     func=LN)
                nc.scalar.activation(out=rb[:, :], in_=lnT[:, :], func=EXP,
                                     scale=-1.0)
                nc.vector.tensor_tensor(
                    resT[0:64, off : off + LN288],
                    avA[0:64, 0:LN288],
                    rb[64:128, :],
                    MULT,
                )
                nc.vector.tensor_tensor(
                    resT[64:128, off : off + LN288],
                    avB[64:128, 0:LN288],
                    rb[0:64, :],
                    MULT,
                )

            return fin

        def make_partial_fin(store, key):
            def fin(av):
                avA, avB = av
                ptl = ptl_pool.tile([128, 2, LN288], F32, tag="ptl", name="ptl", bufs=4)
                nc.vector.tensor_copy(ptl[:, 0, :], avA[:, 0:LN288])
                nc.vector.tensor_copy(ptl[:, 1, :], avB[:, 0:LN288])
                store[key] = ptl

            return fin

        # ------------------------------------------------------------------
        # out-projection stream (one unit behind attention)
        def outproj_quanta(q0, hold):
            quanta = []
            for t in range(NKT):
                def o_q(t=t):
                    w = KW[t]
                    resT = hold["r"]
                    op = ao_psum.tile([128, 512], F32, tag="ao", name="op")
                    nc.tensor.matmul(
                        op[0:w, :],
                        lhsT=resT[:, KOFF[t] : KOFF[t] + w],
                        rhs=wout_sb[:, :],
                        start=True,
                        stop=True,
                    )
                    stg = stage_pool.tile([128, 512], F32, tag="stg", name="stg")
                    nc.vector.tensor_copy(stg[0:w, :], op[0:w, :])
                    nc.sync.dma_start(
                        out=out[q0 + KOFF[t] : q0 + KOFF[t] + w, :],
                        in_=stg[0:w, :],
                    )

                quanta.append(o_q)
            return quanta

        # ------------------------------------------------------------------
        # schedule
        def frame_tiles(u):
            return [(NKT * u + t, S * u + KOFF[t], KW[t]) for t in range(NKT)]

        def group_tiles(frames):
            return [
                (NKT * g + t, S * g + KOFF[t], KW[t])
                for g in frames
                for t in range(NKT)
            ]

        n_groups = 3 if G >= 6 else (2 if G >= 4 else 1)
        cuts = [round(G * i / n_groups) for i in range(n_groups + 1)]
        groups = [list(range(cuts[i], cuts[i + 1])) for i in range(n_groups)]

        if G == 9:
            order = [("S", 1), ("S", 2), ("S", 3), ("T", 0), ("S", 4), ("T", 1),
                     ("S", 5), ("S", 6), ("S", 7), ("T", 2), ("S", 8), ("S", 9),
                     ("S", 10), ("S", 11), ("S", 12), ("S", 13), ("S", 14),
                     ("S", 15)]
            # latest-possible projection placement keeps PE-dense proj work
            # flowing through the tail units so the HAM clock gate stays
            # warm; the bare units are the ACT-rich temporal sub-units
            unit_projs = [[2], [3], [0], [4], [5], [], [6], [7], [8], [],
                          [9], [10], [11], [12], [13], [14], [15], []]
        else:
            # safe fallback: temporal sub-units at the end
            order = [("S", u) for u in range(1, F)] + [
                ("T", k) for k in range(n_groups)
            ]
            unit_projs = [[0, 2]] + [[f] for f in range(3, F)] + [
                [] for _ in range(len(order) - (F - 2))
            ]

        # prologue: frame 1 first so attention starts early
        _weave([proj_quanta(1, pre_xt=xt1)])

        ptl_store = {}
        holdT = {}
        op_queue = []
        for idx, (kind, k) in enumerate(order):
            is_last = idx == len(order) - 1
            if kind == "S":
                q0 = S * k
                hold = {}
                A = attn_quanta(
                    q0, frame_tiles(k),
                    make_norm_fin(hold, 0),
                    make_norm_fin(hold, QCH[1][0]),
                )
            else:
                q0 = 0
                hold = holdT
                if k < n_groups - 1:
                    A = attn_quanta(
                        q0, group_tiles(groups[k]),
                        make_partial_fin(ptl_store, (k, 0)),
                        make_partial_fin(ptl_store, (k, 1)),
                    )
                else:
                    def parts_for(c):
                        return lambda: [
                            ptl_store[(j, c)] for j in range(n_groups - 1)
                        ]

                    A = attn_quanta(
                        q0, group_tiles(groups[k]),
                        make_norm_fin(holdT, 0, parts_for(0)),
                        make_norm_fin(holdT, QCH[1][0], parts_for(1)),
                    )

            if is_last:
                # inline this unit's out-projection: tiles 0-1 only need the
                # chunk-0 half of resT, tiles 2-4 the full row
                oq = outproj_quanta(q0, hold)
                nkl = NKT if kind == "S" else NKT * len(groups[k])
                avs1 = A[-(nkl + 1):-1]
                fin1q = A[-1]
                head = A[:-(nkl + 1)]
                A = (head + [avs1[0], oq[0], avs1[1], oq[1]] + avs1[2:]
                     + [fin1q] + oq[2:])

            P = []
            for pf in unit_projs[idx]:
                P.extend(proj_quanta(pf))
            O = []
            if op_queue:
                O = outproj_quanta(*op_queue.pop(0))
            _weave([A, P, O])

            if not is_last:
                if kind == "S":
                    op_queue.append((q0, hold))
                elif k == n_groups - 1:
                    op_queue.append((0, holdT))

        while op_queue:
            _weave([outproj_quanta(*op_queue.pop(0))])

    import os
    nd = 0  # _dedup_ldweights(nc) — disabled: its PE-array residency model
    # is unsafe under some scheduler interleavings (wrong results with the
    # latest-possible proj schedule); the ~32 NoOp'd loads were worth ~1us
    if os.environ.get("KERNEL_DEBUG"):
        print(f"[kernel] deduped {nd} ldweights")
    _split_drain_waits(nc)
    return nc


_PROG_CACHE = {}


def _get_program(G):
    if G not in _PROG_CACHE:
        _PROG_CACHE[G] = build_program(G)
    return _PROG_CACHE[G]


def _run_spmd(nc, in_maps, trace=False):
    from concourse.bass_utils import run_bass_kernel_spmd

    if trace:
        try:
            from trn_agent_boot.trn_boot import _ntff_profile_via_ctypes

            hook = _ntff_profile_via_ctypes("/opt/axon/libaxon_pjrt.so")
            m = types.ModuleType("antenv.axon_hooks")
            m.get_axon_ntff_profile_hook = lambda: hook
            m.set_axon_ntff_profile_hook = lambda h: None
            sys.modules["antenv.axon_hooks"] = m
        except Exception:
            trace = False
    return run_bass_kernel_spmd(
        nc, in_maps, core_ids=list(range(8)), trace=trace
    )


def _prep(x, drop_mask, Wq, Wk, Wv, Wout):
    import ml_dtypes

    bf16 = ml_dtypes.bfloat16

    dm = np.asarray(drop_mask)
    perms, valid = [], None
    for b in range(B):
        kept = np.nonzero(dm[b] == 0)[0]
        dropped = np.nonzero(dm[b] != 0)[0]
        if valid is None:
            valid = len(kept)
        assert len(kept) == valid, "drop_mask rows must keep equal counts"
        perm = np.concatenate(
            [np.array([0, 1], dtype=np.int64), kept + 2, dropped + 2]
        )
        perms.append(perm)
    G = 2 + valid

    x = np.asarray(x, dtype=np.float32)
    xTs = []
    for b in range(B):
        xt = np.ascontiguousarray(
            x[b].transpose(2, 1, 0)[:, perms[b], :].reshape(D, NT)
        ).astype(bf16)
        xTs.append(xt)
    Wq = (np.asarray(Wq, np.float32) * (1.0 / np.sqrt(C))).astype(bf16)
    Wk = np.asarray(Wk, np.float32).astype(bf16)
    Wv = np.asarray(Wv, np.float32).astype(bf16)
    Wout = np.asarray(Wout, np.float32).astype(bf16)

    in_maps = []
    for core in range(8):
        b, hp = core // 4, core % 4
        sl = slice(128 * hp, 128 * (hp + 1))
        in_maps.append(
            {
                "xT": xTs[b],
                "wq": np.ascontiguousarray(Wq[:, sl]),
                "wk": np.ascontiguousarray(Wk[:, sl]),
                "wv": np.ascontiguousarray(Wv[:, sl]),
                "wout": np.ascontiguousarray(Wout[sl, :]),
            }
        )
    return G, perms, in_maps


def _gather(results, perms, bout):
    bout = np.asarray(bout, np.float32)
    out = np.empty((B, S, F, D), np.float32)
    for b in range(B):
        part = results[4 * b]["out"].astype(np.float32)
        for i in range(1, 4):
            part = part + results[4 * b + i]["out"]
        fsd = part.reshape(F, S, D)
        orig = np.empty_like(fsd)
        orig[perms[b]] = fsd
        out[b] = orig.transpose(1, 0, 2) + bout
    return out


def kernel_traced(x, drop_mask, Wq, Wk, Wv, Wout, bout, trace=False):
    G, perms, in_maps = _prep(x, drop_mask, Wq, Wk, Wv, Wout)
    nc = _get_program(G)
    res = _run_spmd(nc, in_maps, trace=trace)
    return _gather(res.results, perms, bout), res


def kernel(x, drop_mask, Wq, Wk, Wv, Wout, bout):
    out, _ = kernel_traced(x, drop_mask, Wq, Wk, Wv, Wout, bout, trace=False)
    return out


# revision 16
# speedup vs baseline: 1.0192x; 1.0192x over previous
"""Trainium2 Bass kernel for nn_CrossAttentionTemporal3D (v2).

Sharding: batch x head-pair across 8 cores (core c -> batch c//4, heads
{2*(c%4), 2*(c%4)+1}).  Each core computes q/k/v projections for its two
heads, per-frame spatial attention (frames 1..15) and frame-0 temporal
attention, then the out-projection partial product for its 128 hc
columns.  Host sums the 4 partial outputs per batch and adds bout.

v3 structural changes vs the 317us baseline:
  * QK emitted chunk-inner (per key tile: c0h0, c0h1, c1h0, c1h1), and a
    post-schedule pass NoOps redundant InstLdweights: a load whose
    (weights AP, tile_position) is already resident in its row strips
    with no intervening conflicting load.  Halves QK weight-load
    traffic (LDW ~96ns was pacing the PE against 120ns streams).
  * Norm: the two Ln reads stay per-av-tile, but the two Exp(-x) merge
    into ONE 288-col pass over the shared lnT tile (the Lns write
    disjoint partition halves).  ACT per chunk 1764ns -> ~1190ns.
  * Temporal attention split into 3 sub-units of 3 key frames each,
    placed between spatial units so its exp-heavy (ACT-bound) work is
    spread across the kernel instead of bursting.  Sub-unit partial
    sums are staged to SBUF and merged with PSUM RMW adds before the
    final norm.
  * Projections flow 1-per-unit through position 14 (instead of 1/unit
    then a 5-frame burst at the temporal unit), keeping the PE busy in
    the tail so the HAM clock gate stays at 2.4 GHz (baseline lost
    ~39us to K=4/8 oscillation after t=210us).
  * Frame 1 is projected first so attention/ACT work starts ~2us in.
  * PSUM layout as baseline: lg 2x[128,2,512] (4 banks) + ao rotation
    4x[128,512] (avA/avB/pp/op share 4 slots).

Token layout on device: frame-major (token = f*576 + s) with frames
permuted so the temporal key frames [0, 1, kept...] come first.  Host
pre-transposes x to xT [512, 9216] bf16 in that order.
"""

import sys
import types

for _p in (
    "/root/.axon_site",
    "/root/.axon_site/_ro/trn_rl_repo",
    "/root/.axon_site/_ro/pypackages",
    "/opt/trn_rl_repo",
    "/opt/pypackages",
):
    if _p not in sys.path:
        sys.path.append(_p)

import numpy as np

import concourse.bass as bass
import concourse.tile as tile
from concourse import mybir

F32 = mybir.dt.float32
BF16 = mybir.dt.bfloat16

B, S, F, D = 2, 576, 16, 512
H, C = 8, 64
NT = S * F          # 9216 tokens per batch (frame-major)
NKT = 5             # key tiles per frame: 4 full + one 64-tail
KW = [128, 128, 128, 128, 64]
KOFF = [0, 128, 256, 384, 512]
QCH = [(0, 288), (288, 288)]  # query chunks (offset, len)
VTW = 192           # V cols per key tile: [v_h0(64) | 1*64 | v_h1(64)]
LN288 = 288


def _ap_with_free(ap, free_dims):
    """Clone an AP keeping its partition dim, replacing the free dims."""
    return bass.AP(tensor=ap.tensor, offset=ap.offset, ap=[ap.ap[0]] + free_dims)


_WAIT_LIMITS = {k: 1 for k in ("Drain", "Matmult", "Ldweights", "NoOp", "DMACopy", "Activation", "TensorCopy", "TensorTensor", "TensorScalar", "Memset", "ISA", "TensorReduce", "Reciprocal", "DMATransposeAnt", "DmaTransposeAnt", "InstISA")}


def _split_drain_waits(nc):
    """Walrus allows a single sync wait on several opcodes (TPB_CTRL /
    fused S3_LW weight-load).  Hoist extra waits onto one-wait NoOps
    emitted just before the instruction on the same engine."""
    for bb in nc.main_func.blocks:
        new_list, changed = [], False
        for ins in list(bb.instructions):
            si = getattr(ins, "sync_info", None)
            limit = _WAIT_LIMITS.get(ins.opcode)
            if limit is not None and si is not None and len(si.on_wait) > limit:
                waits = list(si.on_wait)
                for i, w in enumerate(waits[limit:]):
                    nop = mybir.InstNoOp(
                        name=f"{ins.name}-wsplit{i}",
                        engine=ins.engine,
                        sync_info=mybir.SyncInfo(on_wait=[w], on_update=[]),
                        bass_nofuse=True,
                    )
                    nc.register_instruction(nop, overwrite=True)
                    new_list.append(nop)
                si.on_wait = waits[:limit]
                changed = True
            new_list.append(ins)
        if changed:
            bb.instructions[:] = new_list


def _dedup_ldweights(nc):
    """NoOp redundant InstLdweights in the scheduled stream: a load whose
    (weights AP, tile_position, tile_size) already sits in its row strips
    with no intervening overlapping load.  The PE array keeps disjoint
    row-group stationaries resident simultaneously (tile_position), so
    the chunk-inner QK pattern [ldwA mm ldwB mm ldwA' mm ldwB' mm]
    re-loads identical weights; ldwA'/ldwB' become NoOps (later bacc
    passes fuse/remove waitless NoOps entirely)."""
    n = 0
    for bb in nc.main_func.blocks:
        resident = []  # list of (row_lo, row_hi, sig)
        new_list = []
        for ins in list(bb.instructions):
            if ins.opcode == "Ldweights":
                tp = ins.tile_position or (0, 0)
                ts = ins.tile_size
                rows = ts[0] if ts else 128
                lo, hi = tp[0], tp[0] + rows
                sig = (str(ins.ins[0]), tuple(tp), tuple(ts) if ts else None,
                       ins.perf_mode, ins.is_transpose)
                hit = any(lo == rl and hi == rh and sig == rs
                          for rl, rh, rs in resident)
                if hit:
                    nop = mybir.InstNoOp(
                        name=ins.name,
                        engine=ins.engine,
                        sync_info=ins.sync_info,
                        bass_nofuse=True,
                    )
                    nc.register_instruction(nop, overwrite=True)
                    new_list.append(nop)
                    n += 1
                    continue
                resident = [(rl, rh, rs) for rl, rh, rs in resident
                            if rh <= lo or rl >= hi]
                resident.append((lo, hi, sig))
            new_list.append(ins)
        bb.instructions[:] = new_list
    return n


def _weave(streams):
    """Interleave quanta lists proportionally (lowest emitted-fraction
    first), preserving each stream's internal order."""
    streams = [s for s in streams if s]
    idx = [0] * len(streams)
    total = sum(len(s) for s in streams)
    for _ in range(total):
        best, bf = -1, 10.0
        for i, s in enumerate(streams):
            if idx[i] < len(s):
                f = (idx[i] + 1.0) / len(s)
                if f < bf:
                    bf, best = f, i
        streams[best][idx[best]]()
        idx[best] += 1


def build_program(G):
    """Build the per-core Bass program. G = number of temporal key frames."""
    nc = bass.Bass()
    xT = nc.dram_tensor("xT", [D, NT], BF16, kind="ExternalInput")
    wq = nc.dram_tensor("wq", [D, 128], BF16, kind="ExternalInput")
    wk = nc.dram_tensor("wk", [D, 128], BF16, kind="ExternalInput")
    wv = nc.dram_tensor("wv", [D, 128], BF16, kind="ExternalInput")
    wout = nc.dram_tensor("wout", [128, D], BF16, kind="ExternalInput")
    out = nc.dram_tensor("out", [NT, D], F32, kind="ExternalOutput")

    NOSYNC = mybir.DependencyInfo.NO_SYNC_ONLY
    EXP = mybir.ActivationFunctionType.Exp
    LN = mybir.ActivationFunctionType.Ln
    MULT = mybir.AluOpType.mult
    ADD = mybir.AluOpType.add

    from contextlib import ExitStack

    with tile.TileContext(nc) as tc, ExitStack() as ctx:
        consts = ctx.enter_context(tc.tile_pool(name="consts", bufs=1))
        big = ctx.enter_context(tc.tile_pool(name="big", bufs=1))
        xt_pool = ctx.enter_context(tc.tile_pool(name="xt", bufs=3))
        pt_pool = ctx.enter_context(tc.tile_pool(name="pt", bufs=34))
        resT_pool = ctx.enter_context(tc.tile_pool(name="resT", bufs=3))
        nrm_pool = ctx.enter_context(tc.tile_pool(name="nrm", bufs=2))
        stage_pool = ctx.enter_context(tc.tile_pool(name="stg", bufs=3))
        ptl_pool = ctx.enter_context(tc.tile_pool(name="ptl", bufs=4))
        lg_psum = ctx.enter_context(tc.tile_pool(name="lg", bufs=2, space="PSUM"))
        ao_psum = ctx.enter_context(tc.tile_pool(name="ao", bufs=4, space="PSUM"))

        # ---- frame-1 tokens first: the whole pipeline's critical path
        # starts at proj(1), so its DMA goes ahead of the constants.
        def xt_dma(f):
            # split per dt4 block so the first projection matmul can start
            # as soon as the first quarter lands
            xt = xt_pool.tile([128, 4, S], BF16, tag="xt", name="xt")
            src = xT.rearrange("(a p) n -> p a n", p=128)[:, :, S * f : S * (f + 1)]
            for j in range(4):
                nc.sync.dma_start(out=xt[:, j, :], in_=src[:, j, :])
            return xt

        xt1 = xt_dma(1)

        # ---- constants
        wq_sb = consts.tile([128, 4, 128], BF16)
        wk_sb = consts.tile([128, 4, 128], BF16)
        wv_sb = consts.tile([128, 4, 128], BF16)
        wout_sb = consts.tile([128, 512], BF16)
        nc.sync.dma_start(out=wq_sb, in_=wq.rearrange("(a p) c -> p a c", p=128))
        nc.sync.dma_start(out=wk_sb, in_=wk.rearrange("(a p) c -> p a c", p=128))
        nc.sync.dma_start(out=wv_sb, in_=wv.rearrange("(a p) c -> p a c", p=128))
        nc.sync.dma_start(out=wout_sb, in_=wout[:, :])

        # ---- HAM warm-up: ~4us of dependency-free junk matmuls that run
        # during the initial DMA-wait window (PE idle 6.5-13us otherwise),
        # flipping the clock gate to 2.4GHz before the first projection.
        # Values are garbage and never read; the lg slot is recycled by the
        # pool semaphores before its first real use.
        junk = consts.tile([128, 512], BF16, name="junk")
        nc.gpsimd.memset(junk[:, :], 1.0)
        jp = lg_psum.tile([128, 2, 512], F32, tag="lg", name="jp")
        for _ in range(12):
            nc.tensor.matmul(
                jp[:, 0, :],
                lhsT=junk[:, 0:128],
                rhs=junk[:, :],
                start=True,
                stop=True,
            )

        # ---- persistent activations
        qT = big.tile([128, NT], BF16)   # [2-head c, token]
        kT = big.tile([128, NT], BF16)
        V = big.tile([128, F * NKT * VTW], BF16)
        # ones columns 64:128 of every key tile
        nc.vector.memset(
            _ap_with_free(V[:, 64:65], [[VTW, F * NKT], [1, 64]]), 1.0
        )

        # ------------------------------------------------------------------
        # projection stream for frame f: dma, q/k chunks, v token-tiles
        def proj_quanta(f, pre_xt=None):
            st = {}
            quanta = []

            def dma_q():
                st["xt"] = pre_xt if pre_xt is not None else xt_dma(f)

            quanta.append(dma_q)

            for w_sb, dest in ((wq_sb, qT), (wk_sb, kT)):
                for off, ln in QCH:
                    def qk_q(w_sb=w_sb, dest=dest, off=off, ln=ln):
                        pp = ao_psum.tile([128, 512], F32, tag="ao", name="pp")
                        for dt4 in range(4):
                            nc.tensor.matmul(
                                pp[:, 0:ln],
                                lhsT=w_sb[:, dt4, :],
                                rhs=st["xt"][:, dt4, off : off + ln],
                                start=(dt4 == 0),
                                stop=(dt4 == 3),
                            )
                        nc.vector.tensor_copy(
                            dest[:, S * f + off : S * f + off + ln],
                            pp[:, 0:ln],
                        )

                    quanta.append(qk_q)

            for t in range(NKT):
                def v_q(t=t):
                    w = KW[t]
                    pp = ao_psum.tile([128, 512], F32, tag="ao", name="ppv")
                    for dt4 in range(4):
                        nc.tensor.matmul(
                            pp[0:w, 0:128],
                            lhsT=st["xt"][:, dt4, KOFF[t] : KOFF[t] + w],
                            rhs=wv_sb[:, dt4, :],
                            start=(dt4 == 0),
                            stop=(dt4 == 3),
                        )
                    base = VTW * (NKT * f + t)
                    dst = _ap_with_free(V[0:w, base : base + 1], [[128, 2], [1, 64]])
                    src = _ap_with_free(pp[0:w, 0:1], [[64, 2], [1, 64]])
                    nc.vector.tensor_copy(out=dst, in_=src)

                quanta.append(v_q)
            return quanta

        # ------------------------------------------------------------------
        # attention stream: chunk-inner QK with stationary reuse, AV chunk
        # c0 inline, AV chunk c1 deferred to the unit tail.
        def attn_quanta(q0, key_tiles, finish0, finish1):
            nk = len(key_tiles)
            st = {"pts": {}, "lgs": {}, "av": {}}
            quanta = []

            def mk_qk(t):
                def qk(t=t):
                    vt, koff, w = key_tiles[t]
                    mm_w = 128 if koff + 128 <= NT else w
                    lgA = lg_psum.tile([128, 2, 512], F32, tag="lg", name="lgA")
                    lgB = lg_psum.tile([128, 2, 512], F32, tag="lg", name="lgB")
                    for ci, (off, ln) in enumerate(QCH):
                        lg = (lgA, lgB)[ci]
                        for h in (0, 1):
                            hb = 64 * h
                            nc.tensor.matmul(
                                lg[0:mm_w, h, 0:ln],
                                lhsT=kT[hb : hb + 64, koff : koff + mm_w],
                                rhs=qT[hb : hb + 64, q0 + off : q0 + off + ln],
                                start=True,
                                stop=True,
                                tile_position=(hb, 0),
                            )
                    st["lgs"][t] = (lgA, lgB, mm_w)

                return qk

            def mk_exp(t, ci):
                def ex(t=t, ci=ci):
                    lgA, lgB, mm_w = st["lgs"][t]
                    lg = (lgA, lgB)[ci]
                    pt = pt_pool.tile([128, 2, LN288], BF16, tag="pt", name="pt")
                    nc.scalar.activation(
                        out=pt[0:mm_w, :, :],
                        in_=lg[0:mm_w, :, 0:LN288],
                        func=EXP,
                    )
                    st["pts"][(t, ci)] = pt

                return ex

            def mk_av(ci, t):
                def av_q(ci=ci, t=t):
                    vt, koff, w = key_tiles[t]
                    if t == 0:
                        avA = ao_psum.tile([128, 512], F32, tag="ao", name="avA")
                        avB = ao_psum.tile([128, 512], F32, tag="ao", name="avB")
                        st["av"][ci] = (avA, avB)
                    avA, avB = st["av"][ci]
                    pt = st["pts"].pop((t, ci))
                    base = VTW * vt
                    nc.tensor.matmul(
                        avA[0:128, 0:LN288],
                        lhsT=V[0:w, base : base + 128],
                        rhs=pt[0:w, 0, :],
                        start=(t == 0),
                        stop=(t == nk - 1),
                    )
                    nc.tensor.matmul(
                        avB[0:128, 0:LN288],
                        lhsT=V[0:w, base + 64 : base + VTW],
                        rhs=pt[0:w, 1, :],
                        start=(t == 0),
                        stop=(t == nk - 1),
                    )

                return av_q

            quanta.append(mk_qk(0))
            if nk > 1:
                quanta.append(mk_qk(1))
            for t in range(nk):
                quanta.append(mk_exp(t, 0))
                quanta.append(mk_av(0, t))
                quanta.append(mk_exp(t, 1))
                if t + 2 < nk:
                    quanta.append(mk_qk(t + 2))
            quanta.append(lambda: finish0(st["av"][0]))
            for t in range(nk):
                quanta.append(mk_av(1, t))
            quanta.append(lambda: finish1(st["av"][1]))
            return quanta

        # ---- chunk finishers
        def make_norm_fin(hold, off, parts=None):
            def fin(av):
                avA, avB = av
                if "r" not in hold:
                    hold["r"] = resT_pool.tile(
                        [128, S], BF16, tag="resT", name="resT"
                    )
                resT = hold["r"]
                if parts is not None:
                    for p in parts():
                        nc.vector.tensor_tensor(
                            avA[:, 0:LN288], avA[:, 0:LN288], p[:, 0, :], ADD
                        )
                        nc.vector.tensor_tensor(
                            avB[:, 0:LN288], avB[:, 0:LN288], p[:, 1, :], ADD
                        )
                lnT = nrm_pool.tile([128, LN288], F32, tag="lnT", name="lnT")
                rb = nrm_pool.tile([128, LN288], F32, tag="rb", name="rb")
                nc.scalar.activation(out=lnT[64:128, :], in_=avA[64:128, 0:LN288],
                                     func=LN)
                nc.scalar.activation(out=lnT[0:64, :], in_=avB[0:64, 0:LN288],
                                     func=LN)
                nc.scalar.activation(out=rb[:, :], in_=lnT[:, :], func=EXP,
                                     scale=-1.0)
                nc.vector.tensor_tensor(
                    resT[0:64, off : off + LN288],
                    avA[0:64, 0:LN288],
                    rb[64:128, :],
                    MULT,
                )
                nc.vector.tensor_tensor(
                    resT[64:128, off : off + LN288],
                    avB[64:128, 0:LN288],
                    rb[0:64, :],
                    MULT,
                )

            return fin

        def make_partial_fin(store, key):
            def fin(av):
                avA, avB = av
                ptl = ptl_pool.tile([128, 2, LN288], F32, tag="ptl", name="ptl", bufs=4)
                nc.vector.tensor_copy(ptl[:, 0, :], avA[:, 0:LN288])
                nc.vector.tensor_copy(ptl[:, 1, :], avB[:, 0:LN288])
                store[key] = ptl

            return fin

        # ------------------------------------------------------------------
        # out-projection stream (one unit behind attention)
        def outproj_quanta(q0, hold):
            quanta = []
            for t in range(NKT):
                def o_q(t=t):
                    w = KW[t]
                    resT = hold["r"]
                    op = ao_psum.tile([128, 512], F32, tag="ao", name="op")
                    nc.tensor.matmul(
                        op[0:w, :],
                        lhsT=resT[:, KOFF[t] : KOFF[t] + w],
                        rhs=wout_sb[:, :],
                        start=True,
                        stop=True,
                    )
                    stg = stage_pool.tile([128, 512], F32, tag="stg", name="stg")
                    nc.vector.tensor_copy(stg[0:w, :], op[0:w, :])
                    nc.sync.dma_start(
                        out=out[q0 + KOFF[t] : q0 + KOFF[t] + w, :],
                        in_=stg[0:w, :],
                    )

                quanta.append(o_q)
            return quanta

        # ------------------------------------------------------------------
        # schedule
        def frame_tiles(u):
            return [(NKT * u + t, S * u + KOFF[t], KW[t]) for t in range(NKT)]

        def group_tiles(frames):
            return [
                (NKT * g + t, S * g + KOFF[t], KW[t])
                for g in frames
                for t in range(NKT)
            ]

        n_groups = 3 if G >= 6 else (2 if G >= 4 else 1)
        cuts = [round(G * i / n_groups) for i in range(n_groups + 1)]
        groups = [list(range(cuts[i], cuts[i + 1])) for i in range(n_groups)]

        if G == 9:
            order = [("S", 1), ("S", 2), ("S", 3), ("T", 0), ("S", 4), ("T", 1),
                     ("S", 5), ("S", 6), ("S", 7), ("T", 2), ("S", 8), ("S", 9),
                     ("S", 10), ("S", 11), ("S", 12), ("S", 13), ("S", 14),
                     ("S", 15)]
            # latest-possible projection placement keeps PE-dense proj work
            # flowing through the tail units so the HAM clock gate stays
            # warm; the bare units are the ACT-rich temporal sub-units
            unit_projs = [[2], [3], [0], [4], [5], [], [6], [7], [8], [],
                          [9], [10], [11], [12], [13], [14], [15], []]
        else:
            # safe fallback: temporal sub-units at the end
            order = [("S", u) for u in range(1, F)] + [
                ("T", k) for k in range(n_groups)
            ]
            unit_projs = [[0, 2]] + [[f] for f in range(3, F)] + [
                [] for _ in range(len(order) - (F - 2))
            ]

        # prologue: frame 1 first so attention starts early
        _weave([proj_quanta(1, pre_xt=xt1)])

        ptl_store = {}
        holdT = {}
        op_queue = []
        for idx, (kind, k) in enumerate(order):
            is_last = idx == len(order) - 1
            if kind == "S":
                q0 = S * k
                hold = {}
                A = attn_quanta(
                    q0, frame_tiles(k),
                    make_norm_fin(hold, 0),
                    make_norm_fin(hold, QCH[1][0]),
                )
            else:
                q0 = 0
                hold = holdT
                if k < n_groups - 1:
                    A = attn_quanta(
                        q0, group_tiles(groups[k]),
                        make_partial_fin(ptl_store, (k, 0)),
                        make_partial_fin(ptl_store, (k, 1)),
                    )
                else:
                    def parts_for(c):
                        return lambda: [
                            ptl_store[(j, c)] for j in range(n_groups - 1)
                        ]

                    A = attn_quanta(
                        q0, group_tiles(groups[k]),
                        make_norm_fin(holdT, 0, parts_for(0)),
                        make_norm_fin(holdT, QCH[1][0], parts_for(1)),
                    )

            if is_last:
                # inline this unit's out-projection: tiles 0-1 only need the
                # chunk-0 half of resT, tiles 2-4 the full row
                oq = outproj_quanta(q0, hold)
                nkl = NKT if kind == "S" else NKT * len(groups[k])
                avs1 = A[-(nkl + 1):-1]
                fin1q = A[-1]
                head = A[:-(nkl + 1)]

                # junk matmuls between the tail out-proj quanta keep the HAM
                # clock gate at 2.4GHz through the drain (PE density drops
                # while the final copy->DMA chains retire, else the last ops
                # run at the 1.2GHz cold clock)
                def mk_junk(n):
                    def jq(n=n):
                        jp2 = lg_psum.tile(
                            [128, 2, 512], F32, tag="lg", name="jp2"
                        )
                        for _ in range(n):
                            nc.tensor.matmul(
                                jp2[:, 0, :],
                                lhsT=junk[:, 0:128],
                                rhs=junk[:, :],
                                start=True,
                                stop=True,
                            )
                    return jq

                A = (head + [avs1[0], oq[0], avs1[1], oq[1]] + avs1[2:]
                     + [fin1q, oq[2], mk_junk(4), oq[3], mk_junk(4), oq[4],
                        mk_junk(10)])

            P = []
            for pf in unit_projs[idx]:
                P.extend(proj_quanta(pf))
            O = []
            if op_queue:
                O = outproj_quanta(*op_queue.pop(0))
            _weave([A, P, O])

            if not is_last:
                if kind == "S":
                    op_queue.append((q0, hold))
                elif k == n_groups - 1:
                    op_queue.append((0, holdT))

        while op_queue:
            _weave([outproj_quanta(*op_queue.pop(0))])

    import os
    nd = 0  # _dedup_ldweights(nc) — disabled: its PE-array residency model
    # is unsafe under some scheduler interleavings (wrong results with the
    # latest-possible proj schedule); the ~32 NoOp'd loads were worth ~1us
    if os.environ.get("KERNEL_DEBUG"):
        print(f"[kernel] deduped {nd} ldweights")
    _split_drain_waits(nc)
    return nc


_PROG_CACHE = {}


def _get_program(G):
    if G not in _PROG_CACHE:
        _PROG_CACHE[G] = build_program(G)
    return _PROG_CACHE[G]


def _run_spmd(nc, in_maps, trace=False):
    from concourse.bass_utils import run_bass_kernel_spmd

    if trace:
        try:
            from trn_agent_boot.trn_boot import _ntff_profile_via_ctypes

            hook = _ntff_profile_via_ctypes("/opt/axon/libaxon_pjrt.so")
            m = types.ModuleType("antenv.axon_hooks")
            m.get_axon_ntff_profile_hook = lambda: hook
            m.set_axon_ntff_profile_hook = lambda h: None
            sys.modules["antenv.axon_hooks"] = m
        except Exception:
            trace = False
    return run_bass_kernel_spmd(
        nc, in_maps, core_ids=list(range(8)), trace=trace
    )


def _prep(x, drop_mask, Wq, Wk, Wv, Wout):
    import ml_dtypes

    bf16 = ml_dtypes.bfloat16

    dm = np.asarray(drop_mask)
    perms, valid = [], None
    for b in range(B):
        kept = np.nonzero(dm[b] == 0)[0]
        dropped = np.nonzero(dm[b] != 0)[0]
        if valid is None:
            valid = len(kept)
        assert len(kept) == valid, "drop_mask rows must keep equal counts"
        perm = np.concatenate(
            [np.array([0, 1], dtype=np.int64), kept + 2, dropped + 2]
        )
        perms.append(perm)
    G = 2 + valid

    x = np.asarray(x, dtype=np.float32)
    xTs = []
    for b in range(B):
        xt = np.ascontiguousarray(
            x[b].transpose(2, 1, 0)[:, perms[b], :].reshape(D, NT)
        ).astype(bf16)
        xTs.append(xt)
    Wq = (np.asarray(Wq, np.float32) * (1.0 / np.sqrt(C))).astype(bf16)
    Wk = np.asarray(Wk, np.float32).astype(bf16)
    Wv = np.asarray(Wv, np.float32).astype(bf16)
    Wout = np.asarray(Wout, np.float32).astype(bf16)

    in_maps = []
    for core in range(8):
        b, hp = core // 4, core % 4
        sl = slice(128 * hp, 128 * (hp + 1))
        in_maps.append(
            {
                "xT": xTs[b],
                "wq": np.ascontiguousarray(Wq[:, sl]),
                "wk": np.ascontiguousarray(Wk[:, sl]),
                "wv": np.ascontiguousarray(Wv[:, sl]),
                "wout": np.ascontiguousarray(Wout[sl, :]),
            }
        )
    return G, perms, in_maps


def _gather(results, perms, bout):
    bout = np.asarray(bout, np.float32)
    out = np.empty((B, S, F, D), np.float32)
    for b in range(B):
        part = results[4 * b]["out"].astype(np.float32)
        for i in range(1, 4):
            part = part + results[4 * b + i]["out"]
        fsd = part.reshape(F, S, D)
        orig = np.empty_like(fsd)
        orig[perms[b]] = fsd
        out[b] = orig.transpose(1, 0, 2) + bout
    return out


def kernel_traced(x, drop_mask, Wq, Wk, Wv, Wout, bout, trace=False):
    G, perms, in_maps = _prep(x, drop_mask, Wq, Wk, Wv, Wout)
    nc = _get_program(G)
    res = _run_spmd(nc, in_maps, trace=trace)
    return _gather(res.results, perms, bout), res


def kernel(x, drop_mask, Wq, Wk, Wv, Wout, bout):
    out, _ = kernel_traced(x, drop_mask, Wq, Wk, Wv, Wout, bout, trace=False)
    return out
